# revision 26
# baseline (speedup 1.0000x reference)
"""AttnBlock (GroupNorm -> single-head attention over 4096 tokens -> proj
-> residual) on 8 Trainium2 NeuronCores.

Sharding: batch (4) x query-token-half (2) = 8 cores, no collectives.
Each core receives its batch's full x (fp16), ROLLED along the token axis so
that its 2048 query tokens are always tokens [0:2048] — attention and the
group statistics are permutation-invariant over key tokens, so every core
runs the identical program. The core computes group-norm stats over all 4096
tokens, K and V^T for all tokens, Q for its 2048 queries, attention + proj
for those queries, and returns the fp16 delta (proj output + proj bias).
The residual add x + delta happens on the host in fp32.

On-core layouts (c = channel, t/j = key token, i = query token):
  hn, k:   [c, t]  (channels on partitions)  -> scoresT = k^T q directly
  vT:      [t, c]  (tokens on partitions)    -> AV contracts j on partitions
  scoresT: [j, i]  softmax denom via DVE adds + replicated-ones matmul bcast
All matmul operands are fp16 (PE full rate, fp32 PSUM accumulation).
V's bias is folded in after normalization (out/Z + bv) since softmax weights
sum to one, and the softmax max-subtraction is skipped (scores ~ N(0,1)).
"""
import numpy as np

C = 512
N_TOK = 4096
HALF = 2048
B = 4
N_CORES = 8
NUM_GROUPS = 32
EPS = 1e-6
SCALE = float(C) ** -0.5
GROUP_N = (C // NUM_GROUPS) * N_TOK

_CACHE = {}


def _build_nc():
    from contextlib import ExitStack

    import concourse.bass as bass
    import concourse.mybir as mybir
    import concourse.tile as tile

    f32 = mybir.dt.float32
    f16 = mybir.dt.float16
    AF = mybir.ActivationFunctionType
    ALU = mybir.AluOpType
    AX = mybir.AxisListType

    nc = bass.Bass()
    x_ext = nc.declare_dram_parameter("x", [C, N_TOK], f16, isOutput=False)
    wqT_ext = nc.declare_dram_parameter("wqT", [C, C], f16, isOutput=False)
    wkT_ext = nc.declare_dram_parameter("wkT", [C, C], f16, isOutput=False)
    wvT_ext = nc.declare_dram_parameter("wvT", [C, C], f16, isOutput=False)
    wpT_ext = nc.declare_dram_parameter("wpT", [C, C], f16, isOutput=False)
    bq_ext = nc.declare_dram_parameter("bq", [C], f32, isOutput=False)
    bk_ext = nc.declare_dram_parameter("bk", [C], f32, isOutput=False)
    bv_ext = nc.declare_dram_parameter("bv", [C], f32, isOutput=False)
    bp_ext = nc.declare_dram_parameter("bp", [C], f32, isOutput=False)
    gam_ext = nc.declare_dram_parameter("gamma", [C], f32, isOutput=False)
    bet_ext = nc.declare_dram_parameter("beta", [C], f32, isOutput=False)
    sel_ext = nc.declare_dram_parameter("sel", [128, 8], f32, isOutput=False)
    selT_ext = nc.declare_dram_parameter("selT", [8, 128], f32, isOutput=False)
    ones_ext = nc.declare_dram_parameter("ones", [128, 128], f32, isOutput=False)
    yd_ext = nc.declare_dram_parameter("yd", [C, HALF], f16, isOutput=True)

    x_r = x_ext.rearrange("(ct p) n -> p ct n", p=128)
    yd_r = yd_ext.rearrange("(ct p) n -> p ct n", p=128)

    with tile.TileContext(nc) as tc, ExitStack() as top:
        consts = top.enter_context(tc.tile_pool(name="consts", bufs=1))
        big = top.enter_context(tc.tile_pool(name="big", bufs=1))

        k_lo = big.tile([128, 4, HALF], f16, name="k_lo")
        k_hi = big.tile([128, 4, HALF], f16, name="k_hi")
        vT_lo = big.tile([128, 16, C], f16, name="vT_lo")
        vT_hi = big.tile([128, 16, C], f16, name="vT_hi")
        q_sb = big.tile([128, 4, HALF], f16, name="q_sb")
        xfull = big.tile([128, 4, N_TOK], f16, name="xfull")
        wp_sb = big.tile([128, 4, C], f16, name="wp_sb")
        nc.sync.dma_start(out=wp_sb, in_=wpT_ext.rearrange("(ci p) o -> p ci o", p=128))

        sel_sb = consts.tile([128, 8], f32, name="sel_sb")
        nc.sync.dma_start(out=sel_sb, in_=sel_ext[:])
        selT_sb = consts.tile([8, 128], f32, name="selT_sb")
        nc.sync.dma_start(out=selT_sb, in_=selT_ext[:])
        ones_sb = consts.tile([128, 128], f32, name="ones_sb")
        nc.sync.dma_start(out=ones_sb, in_=ones_ext[:])
        ones16 = consts.tile([128, 128], f16, name="ones16")
        nc.vector.tensor_copy(ones16, ones_sb)

        def load_bias(name, ext):
            t = consts.tile([128, 4], f32, name=name)
            nc.sync.dma_start(out=t, in_=ext.rearrange("(ct p) -> p ct", p=128))
            return t

        bq_sb = load_bias("bq_sb", bq_ext)
        bk_sb = load_bias("bk_sb", bk_ext)
        bv_sb = load_bias("bv_sb", bv_ext)
        bp_sb = load_bias("bp_sb", bp_ext)
        gam_sb = load_bias("gam_sb", gam_ext)
        bet_sb = load_bias("bet_sb", bet_ext)

        # ---------------- Stage A: group-norm statistics ----------------
        sc = consts.tile([128, 4], f32, name="sc")
        bi = consts.tile([128, 4], f32, name="bi")
        with ExitStack() as stA:
            pa = stA.enter_context(tc.tile_pool(name="pa", bufs=2))
            psA = stA.enter_context(tc.tile_pool(name="psA", bufs=2, space="PSUM"))
            sums = consts.tile([128, 4, 2, 2], f32, name="sums")
            for ct in range(4):
                for h in range(2):
                    hs = slice(h * HALF, (h + 1) * HALF)
                    nc.sync.dma_start(out=xfull[:, ct, hs],
                                      in_=x_ext[ct * 128:(ct + 1) * 128, hs])
                    sq = pa.tile([128, HALF], f16, name="sq")
                    nc.scalar.activation(out=sq, in_=xfull[:, ct, hs], func=AF.Square,
                                         accum_out=sums[:, ct, h, 1:2])
                    nc.vector.tensor_reduce(out=sums[:, ct, h, 0:1],
                                            in_=xfull[:, ct, hs],
                                            axis=AX.X, op=ALU.add)
            gp = psA.tile([8, 8], f32, name="gp")
            for ct in range(4):
                for h in range(2):
                    nc.tensor.matmul(gp[:, ct * 2:(ct + 1) * 2], sel_sb,
                                     sums[:, ct, h, :],
                                     start=(h == 0), stop=(h == 1))
            gst3 = consts.tile([8, 4, 2], f32, name="gst3")
            nc.vector.tensor_copy(gst3, gp)
            inv_n = 1.0 / GROUP_N
            m_t = consts.tile([8, 4], f32, name="m_t")
            e2_t = consts.tile([8, 4], f32, name="e2_t")
            nc.vector.tensor_scalar_mul(out=m_t, in0=gst3[:, :, 0], scalar1=inv_n)
            nc.vector.tensor_scalar_mul(out=e2_t, in0=gst3[:, :, 1], scalar1=inv_n)
            msq = consts.tile([8, 4], f32, name="msq")
            nc.vector.tensor_mul(msq, m_t, m_t)
            ve = consts.tile([8, 4], f32, name="ve")
            nc.vector.tensor_tensor(out=ve, in0=e2_t, in1=msq, op=ALU.subtract)
            nc.vector.tensor_scalar_add(ve, ve, EPS)
            sd = consts.tile([8, 4], f32, name="sd")
            nc.scalar.activation(out=sd, in_=ve, func=AF.Sqrt)
            r0 = consts.tile([8, 4], f32, name="r0")
            nc.vector.reciprocal(r0, sd)
            t1 = consts.tile([8, 4], f32, name="t1")
            nc.vector.tensor_mul(t1, r0, r0)
            nc.vector.tensor_mul(t1, t1, ve)
            nc.vector.tensor_scalar(out=t1, in0=t1, scalar1=-0.5, scalar2=1.5,
                                    op0=ALU.mult, op1=ALU.add)
            gmr = consts.tile([8, 4, 2], f32, name="gmr")
            nc.vector.tensor_mul(gmr[:, :, 1], r0, t1)
            nc.vector.tensor_copy(gmr[:, :, 0], m_t)
            chan = consts.tile([128, 4, 2], f32, name="chan")
            for ct in range(4):
                chp = psA.tile([128, 2], f32, name="chp")
                nc.tensor.matmul(chp, selT_sb, gmr[:, ct, :], start=True, stop=True)
                nc.vector.tensor_copy(chan[:, ct, :], chp)
            nc.vector.tensor_tensor(out=sc, in0=chan[:, :, 1], in1=gam_sb, op=ALU.mult)
            tb_ = consts.tile([128, 4], f32, name="tb_")
            nc.vector.tensor_tensor(out=tb_, in0=chan[:, :, 0], in1=sc, op=ALU.mult)
            nc.vector.tensor_tensor(out=bi, in0=bet_sb, in1=tb_, op=ALU.subtract)

        # ---------------- Stage B: k, vT (all tokens), q (queries) ----------
        # Norm is folded into the weights: k = (Wk diag(sc)) x + (bk + Wk bi),
        # so the matmuls consume raw x tiles and no hn is ever materialized.
        # For V the norm-bias term is a per-channel constant, folded into the
        # softmax-normalized output together with bv (weights sum to 1).
        with ExitStack() as stB:
            wts = stB.enter_context(tc.tile_pool(name="wts", bufs=1))
            psB = stB.enter_context(tc.tile_pool(name="psB", bufs=4, space="PSUM"))
            wk_sb = wts.tile([128, 4, C], f16, name="wk_sb")
            nc.sync.dma_start(out=wk_sb, in_=wkT_ext.rearrange("(ci p) o -> p ci o", p=128))
            wv_sb = wts.tile([128, 4, C], f16, name="wv_sb")
            nc.sync.dma_start(out=wv_sb, in_=wvT_ext.rearrange("(ci p) o -> p ci o", p=128))
            wq_sb = wts.tile([128, 4, C], f16, name="wq_sb")
            nc.sync.dma_start(out=wq_sb, in_=wqT_ext.rearrange("(ci p) o -> p ci o", p=128))
            bi16 = consts.tile([128, 4], f16, name="bi16")
            nc.vector.tensor_copy(bi16, bi)
            wkF = wts.tile([128, 4, C], f16, name="wkF")
            wvF = wts.tile([128, 4, C], f16, name="wvF")
            wqF = wts.tile([128, 4, C], f16, name="wqF")
            for ci in range(4):
                nc.vector.tensor_scalar_mul(out=wkF[:, ci, :], in0=wk_sb[:, ci, :],
                                            scalar1=sc[:, ci:ci + 1])
                nc.scalar.activation(out=wvF[:, ci, :], in_=wv_sb[:, ci, :],
                                     func=AF.Identity, scale=sc[:, ci:ci + 1])
                nc.vector.tensor_scalar_mul(out=wqF[:, ci, :], in0=wq_sb[:, ci, :],
                                            scalar1=sc[:, ci:ci + 1])
            # folded bias rows: b' = b + W @ bi   (W original, bi = norm bias)
            bkF = consts.tile([128, 4], f32, name="bkF")
            bqF = consts.tile([128, 4], f32, name="bqF")
            bvv = consts.tile([128, 4], f32, name="bvv")
            for w_sb, b_sb, bF in ((wk_sb, bk_sb, bkF), (wq_sb, bq_sb, bqF),
                                   (wv_sb, bv_sb, bvv)):
                for co in range(4):
                    bp_ps = psB.tile([128, 1], f32, name="bp_ps", tag="bps")
                    for ci in range(4):
                        nc.tensor.matmul(bp_ps, w_sb[:, ci, co * 128:(co + 1) * 128],
                                         bi16[:, ci:ci + 1],
                                         start=(ci == 0), stop=(ci == 3))
                    nc.vector.tensor_tensor(out=bF[:, co:co + 1], in0=bp_ps,
                                            in1=b_sb[:, co:co + 1], op=ALU.add)
            for tb in range(8):
                sl = slice(tb * 512, (tb + 1) * 512)
                for co in range(4):
                    kp = psB.tile([128, 512], f32, name="kp")
                    for ci in range(4):
                        nc.tensor.matmul(kp, wkF[:, ci, co * 128:(co + 1) * 128],
                                         xfull[:, ci, sl], start=(ci == 0), stop=(ci == 3))
                    ksl = slice((tb % 4) * 512, (tb % 4 + 1) * 512)
                    ktgt = k_lo if tb < 4 else k_hi
                    nc.scalar.activation(out=ktgt[:, co, ksl], in_=kp, func=AF.Identity,
                                         bias=bkF[:, co:co + 1])
                for tt in range(4):
                    vp = psB.tile([128, 512], f32, name="vp", tag="kp")
                    for ci in range(4):
                        nc.tensor.matmul(vp, xfull[:, ci, tb * 512 + tt * 128:tb * 512 + (tt + 1) * 128],
                                         wvF[:, ci, :], start=(ci == 0), stop=(ci == 3))
                    vtgt = vT_lo if tb < 4 else vT_hi
                    nc.vector.tensor_copy(vtgt[:, (tb % 4) * 4 + tt, :], vp)
                if tb < 4:
                    for co in range(4):
                        qp = psB.tile([128, 512], f32, name="qp", tag="kp")
                        for ci in range(4):
                            nc.tensor.matmul(qp, wqF[:, ci, co * 128:(co + 1) * 128],
                                             xfull[:, ci, sl], start=(ci == 0), stop=(ci == 3))
                        nc.scalar.activation(out=q_sb[:, co, sl], in_=qp,
                                             func=AF.Identity, bias=bqF[:, co:co + 1])

        # ---------------- Stage C: attention + proj ----------------
        # Software-pipelined emission: scoresT(jg+1) is emitted before AV(jg)
        # so the PE has independent work while ACT computes exp(jg); the
        # previous block's projection is emitted into the next block's
        # prologue so PE covers the PSUM-accumulator handoff.
        with ExitStack() as stC:
            pc = stC.enter_context(tc.tile_pool(name="pc", bufs=2))
            pc2 = stC.enter_context(tc.tile_pool(name="pc2", bufs=3))
            ps_acc = stC.enter_context(tc.tile_pool(name="ps_acc", bufs=1, space="PSUM"))
            ps_sT = stC.enter_context(tc.tile_pool(name="ps_sT", bufs=2, space="PSUM"))

            pending_proj = None
            for ib in range(4):
                isl = slice(ib * 512, (ib + 1) * 512)
                Zp = pc.tile([128, 512], f32, name="Zp")
                nc.vector.memset(Zp, 0.0)
                oap = [None]

                ptgs = {}

                def emit_sT(jg, isl=isl, Zp=Zp, ptgs=ptgs):
                    sTp = ps_sT.tile([128, 2, 512], f32, name="sTp", tag="sT")
                    for jt2 in range(2):
                        jt = jg * 2 + jt2
                        ksrc, jtl = (k_lo, jt) if jt < 16 else (k_hi, jt - 16)
                        for ci in range(4):
                            nc.tensor.matmul(sTp[:, jt2, :],
                                             ksrc[:, ci, jtl * 128:(jtl + 1) * 128],
                                             q_sb[:, ci, isl],
                                             start=(ci == 0), stop=(ci == 3))
                    ptg = pc2.tile([128, 2, 512], f16, name="ptg")
                    nc.scalar.activation(out=ptg, in_=sTp, func=AF.Exp, scale=SCALE)
                    zt = pc2.tile([128, 512], f32, name="zt")
                    nc.vector.tensor_add(zt, ptg[:, 0, :], ptg[:, 1, :])
                    nc.vector.tensor_add(Zp, Zp, zt)
                    ptgs[jg] = ptg

                def emit_AV(jg, ptgs=ptgs, oap=oap):
                    if oap[0] is None:
                        oap[0] = (ps_acc.tile([128, 2, 512], f32, name="oap_a", tag="acc_a"),
                                  ps_acc.tile([128, 2, 512], f32, name="oap_b", tag="acc_b"))
                    ptg = ptgs.pop(jg)
                    for ct in range(4):
                        oap_h = oap[0][0] if ct < 2 else oap[0][1]
                        for jt2 in range(2):
                            jt = jg * 2 + jt2
                            vsrc, jtv = (vT_lo, jt) if jt < 16 else (vT_hi, jt - 16)
                            nc.tensor.matmul(oap_h[:, ct % 2, :],
                                             vsrc[:, jtv, ct * 128:(ct + 1) * 128],
                                             ptg[:, jt2, :],
                                             start=(jg == 0 and jt2 == 0),
                                             stop=(jg == 15 and jt2 == 1))

                emit_sT(0)
                emit_sT(1)
                if pending_proj is not None:
                    pending_proj()
                    pending_proj = None
                for jg in range(16):
                    if jg + 2 < 16:
                        emit_sT(jg + 2)
                    emit_AV(jg)
                Zp16 = pc.tile([128, 512], f16, name="Zp16")
                nc.vector.tensor_copy(Zp16, Zp)
                zbp = ps_sT.tile([128, 512], f32, name="zbp", tag="sT")
                nc.tensor.matmul(zbp, ones16, Zp16, start=True, stop=True)
                rz = pc.tile([128, 512], f32, name="rz")
                nc.vector.reciprocal(rz, zbp)
                oa_sb = pc.tile([128, 4, 512], f16, name="oa_sb")
                for ct in range(4):
                    oap_h = oap[0][0] if ct < 2 else oap[0][1]
                    t1c = pc2.tile([128, 512], f32, name="t1c", tag="zt")
                    nc.vector.tensor_mul(t1c, oap_h[:, ct % 2, :], rz)
                    nc.vector.tensor_scalar(out=oa_sb[:, ct, :], in0=t1c,
                                            scalar1=bvv[:, ct:ct + 1], scalar2=None,
                                            op0=ALU.add)

                def emit_proj(oa_sb=oa_sb, isl=isl):
                    up_a = ps_acc.tile([128, 2, 512], f32, name="up_a", tag="acc_a")
                    up_b = ps_acc.tile([128, 2, 512], f32, name="up_b", tag="acc_b")
                    yd_sb = pc.tile([128, 4, 512], f16, name="yd_sb")
                    for co in range(4):
                        up_h = up_a if co < 2 else up_b
                        for ci in range(4):
                            nc.tensor.matmul(up_h[:, co % 2, :],
                                             wp_sb[:, ci, co * 128:(co + 1) * 128],
                                             oa_sb[:, ci, :],
                                             start=(ci == 0), stop=(ci == 3))
                        nc.vector.tensor_scalar(out=yd_sb[:, co, :],
                                                in0=up_h[:, co % 2, :],
                                                scalar1=bp_sb[:, co:co + 1],
                                                scalar2=None, op0=ALU.add)
                        nc.sync.dma_start(out=yd_r[:, co, isl], in_=yd_sb[:, co, :])

                pending_proj = emit_proj
            pending_proj()

    _split_excess_waits(nc)
    return nc


def _split_excess_waits(nc, limit=1):
    """walrus in this container accepts at most one sync-wait per
    instruction; hoist excess waits onto preceding same-engine NoOps."""
    import bass_rust
    import concourse.mybir as mybir

    n = 0
    for f in nc.m.functions:
        for bb in f.blocks:
            out = []
            for inst in bb.instructions:
                si = inst.sync_info
                waits = list(si.on_wait) if si and si.on_wait else []
                if len(waits) > limit:
                    excess, keep = waits[:-limit], waits[-limit:]
                    for ci in range(0, len(excess), limit):
                        nop = mybir.InstNoOp(name=f"{inst.name}-ws{ci}", ins=[], outs=[])
                        nop.engine = inst.engine
                        nop.sync_info = bass_rust.SyncInfo(
                            on_wait=list(excess[ci:ci + limit]), on_update=[])
                        out.append(nop)
                        n += 1
                    inst.sync_info = bass_rust.SyncInfo(
                        on_wait=list(keep),
                        on_update=list(si.on_update) if si.on_update else [])
                out.append(inst)
            bb.instructions[:] = out
    return n


def _get_nc():
    if "nc" not in _CACHE:
        _CACHE["nc"] = _build_nc()
    return _CACHE["nc"]


def _host_constants():
    sel = np.zeros((128, 8), np.float32)
    for p in range(128):
        sel[p, p // 16] = 1.0
    selT = np.ascontiguousarray(sel.T)
    ones = np.ones((128, 128), np.float32)
    return sel, selT, ones


def _make_in_maps(x, norm_gamma, norm_beta, wq, bq, wk, bk, wv, bv, wp, bp):
    sel, selT, ones = _host_constants()
    common = {
        "wqT": np.ascontiguousarray(np.asarray(wq, np.float32).T.astype(np.float16)),
        "wkT": np.ascontiguousarray(np.asarray(wk, np.float32).T.astype(np.float16)),
        "wvT": np.ascontiguousarray(np.asarray(wv, np.float32).T.astype(np.float16)),
        "wpT": np.ascontiguousarray(np.asarray(wp, np.float32).T.astype(np.float16)),
        "bq": np.asarray(bq, np.float32), "bk": np.asarray(bk, np.float32),
        "bv": np.asarray(bv, np.float32), "bp": np.asarray(bp, np.float32),
        "gamma": np.asarray(norm_gamma, np.float32),
        "beta": np.asarray(norm_beta, np.float32),
        "sel": sel, "selT": selT, "ones": ones,
    }
    in_maps = []
    for core in range(N_CORES):
        b, qh = core // 2, core % 2
        xb = np.asarray(x[b], np.float32).reshape(C, N_TOK).astype(np.float16)
        if qh:
            xb = np.concatenate([xb[:, HALF:], xb[:, :HALF]], axis=1)
        in_maps.append({"x": np.ascontiguousarray(xb), **common})
    return in_maps


def kernel(x, norm_gamma, norm_beta, wq, bq, wk, bk, wv, bv, wp, bp):
    from concourse.bass_utils import run_bass_kernel_spmd

    nc = _get_nc()
    x = np.asarray(x, dtype=np.float32)
    in_maps = _make_in_maps(x, norm_gamma, norm_beta, wq, bq, wk, bk, wv, bv, wp, bp)
    res = run_bass_kernel_spmd(nc, in_maps, list(range(N_CORES)))
    out = np.empty((B, C, N_TOK), np.float32)
    for core in range(N_CORES):
        b, qh = core // 2, core % 2
        sl = slice(qh * HALF, (qh + 1) * HALF)
        out[b, :, sl] = x[b].reshape(C, N_TOK)[:, sl] + \
            res.results[core]["yd"].astype(np.float32)
    return out.reshape(B, C, 64, 64)
